# revision 3
# baseline (speedup 1.0000x reference)
"""Local sparse (sliding-window) attention for Trainium2, 8 NeuronCores.

Problem: q,k,v [4, 4096, 128] f32; window |i-j| <= 32.
Reference returns (output [4,4096,128], attn_weights [4,4096,4096], local_mask [4096,4096] bool).

Sharding (SPMD, one NEFF on 8 cores; all per-core differences are carried in
input *values*, never in code/offsets):
  core c -> batch b = c//2, query rows [base, base+2048) with base = 2048*(c%2).
  Within a 128-row query block m, the +-32 band covers a 192-wide column slab
  at columns base + 128m - 32 ... +160.  Relative to the core's own column
  half [base, base+2048) the slab offset 128m - 32 is core-INDEPENDENT, so:
    - attn_mid [2048, 2048]: the core's near column half (zeros + band slabs).
    - attn_far [2048, 2048]: the other column half: zeros + two 32x32 fringe
      corners (band spill across the half boundary).  Both fringe positions
      are static; the invalid one is exactly zero via the boundary col-mask.
    - out_rows [2048, 128]: attention output rows.
    - mask_rot [512, 4096] u8: local_mask rows [512c, 512c+512), cyclically
      rotated left by (512c - 32) mod 4096 so the band sits at columns [0,576)
      for every core; the host un-rotates with two slice copies.
"""

import os
import sys

import numpy as np

if "/opt/trn_rl_repo" not in sys.path:
    sys.path.insert(0, "/opt/trn_rl_repo")

B, S, D = 4, 4096, 128
HALF = 2048          # rows per core
NBLK = 16            # 128-row blocks per core
KTILES = 17          # padded k/v tiles of 128 rows (2176 rows)
KBAND = KTILES * 128
W = 192              # score slab width
WIN = 32             # half window
NB = 3               # row-tile ring size
SCALE = float(1.0 / np.sqrt(np.float32(D)))
NEG = -1.0e30

_CACHE = {}


# ---------------------------------------------------------------- host shards
def _shards(q, k, v):
    """Per-core input dicts (values differ per core, shapes identical)."""
    maps = []
    for c in range(8):
        b, base = c // 2, HALF * (c % 2)
        kb = np.zeros((KBAND, D), np.float32)
        vb = np.zeros((KBAND, D), np.float32)
        lo = base - WIN
        s_lo, s_hi = max(0, lo), min(S, lo + KBAND)
        kb[s_lo - lo : s_hi - lo] = k[b, s_lo:s_hi]
        vb[s_lo - lo : s_hi - lo] = v[b, s_lo:s_hi]
        # colmask[0/1, w]: additive mask for score slab of block 0 / block 15
        cm = np.zeros((2, W), np.float32)
        j0 = base - WIN + np.arange(W)            # abs col of block-0 slab
        cm[0][(j0 < 0) | (j0 >= S)] = NEG
        j15 = base + 15 * 128 - WIN + np.arange(W)
        cm[1][(j15 < 0) | (j15 >= S)] = NEG
        # maskcol[0/1, w]: validity (1/0) for mask-band blocks tb=0 / tb=3
        mc = np.ones((2, W), np.float32)
        a0 = 512 * c - WIN + np.arange(W)         # rot col w -> abs col
        mc[0][(a0 < 0) | (a0 >= S)] = 0.0
        a3 = 512 * c - WIN + 384 + np.arange(W)
        mc[1][(a3 < 0) | (a3 >= S)] = 0.0
        maps.append(
            {
                "q_rows": np.ascontiguousarray(q[b, base : base + HALF]),
                "k_band": kb,
                "v_band": vb,
                "colmask": cm,
                "maskcol": mc,
            }
        )
    return maps


def _assemble(results):
    out = np.empty((B, S, D), np.float32)
    attn = np.empty((B, S, S), np.float32)
    mask = np.empty((S, S), np.uint8)
    for c in range(8):
        r = results[c]
        b, base = c // 2, HALF * (c % 2)
        far = HALF - base
        out[b, base : base + HALF] = r["out_rows"]
        attn[b, base : base + HALF, base : base + HALF] = r["attn_mid"]
        attn[b, base : base + HALF, far : far + HALF] = r["attn_far"]
        rot = r["mask_rot"]
        a = (512 * c - WIN) % S
        mask[512 * c : 512 * c + 512, a:S] = rot[:, : S - a]
        mask[512 * c : 512 * c + 512, 0:a] = rot[:, S - a :]
    return out, attn, mask.view(np.bool_)


# ------------------------------------------------------- numpy emulation path
def _emulate_core(inp):
    """Numpy model of exactly what one core's device program computes."""
    qr, kb, vb = inp["q_rows"], inp["k_band"], inp["v_band"]
    cm, mc = inp["colmask"], inp["maskcol"]
    p_idx = np.arange(128)[:, None]
    w_idx = np.arange(W)[None, :]
    bandmask = np.where((w_idx - p_idx >= 0) & (w_idx - p_idx - 64 <= 0), 0.0, NEG)
    bandmask = bandmask.astype(np.float32)
    mid = np.zeros((HALF, HALF), np.float32)
    farr = np.zeros((HALF, HALF), np.float32)
    outr = np.empty((HALF, D), np.float32)
    for m in range(NBLK):
        qb = qr[128 * m : 128 * (m + 1)]
        kband = kb[128 * m : 128 * m + W]
        sc = qb @ kband.T
        bm = bandmask.copy()
        if m == 0:
            bm = bm + cm[0]
        if m == 15:
            bm = bm + cm[1]
        s = sc * np.float32(SCALE) + bm
        mx = s.max(axis=1, keepdims=True)
        p = np.exp(s - mx)
        wgt = (p / p.sum(axis=1, keepdims=True)).astype(np.float32)
        outr[128 * m : 128 * (m + 1)] = wgt @ vb[128 * m : 128 * m + W]
        if m == 0:
            mid[0:128, 0:160] = wgt[:, 32:192]
            farr[0:32, 2016:2048] = wgt[0:32, 0:32]
        elif m == 15:
            mid[1920:2048, 1888:2048] = wgt[:, 0:160]
            farr[2016:2048, 0:32] = wgt[96:128, 160:192]
        else:
            mid[128 * m : 128 * (m + 1), 128 * m - 32 : 128 * m + 160] = wgt
    # mask_rot: band pattern at cols [0, 576)
    rot = np.zeros((512, S), np.uint8)
    for tb in range(4):
        pat = ((w_idx - p_idx >= 0) & (w_idx - p_idx - 64 <= 0)).astype(np.uint8)
        if tb == 0:
            pat = pat * (mc[0] != 0)
        if tb == 3:
            pat = pat * (mc[1] != 0)
        rot[128 * tb : 128 * (tb + 1), 128 * tb : 128 * tb + W] = pat
    return {"out_rows": outr, "attn_mid": mid, "attn_far": farr, "mask_rot": rot}


# ------------------------------------------------------------- device program
def _build_nc():
    import concourse.bacc as bacc
    import concourse.bass as bass
    import concourse.mybir as mybir
    import concourse.tile as tile
    from concourse._compat import get_trn_type
    from concourse.masks import make_identity

    FP = mybir.dt.float32
    U8 = mybir.dt.uint8
    ALU = mybir.AluOpType
    AF = mybir.ActivationFunctionType

    nc = bacc.Bacc(get_trn_type() or "TRN2", target_bir_lowering=False, debug=False)
    q_d = nc.dram_tensor("q_rows", [HALF, D], FP, kind="ExternalInput")
    k_d = nc.dram_tensor("k_band", [KBAND, D], FP, kind="ExternalInput")
    v_d = nc.dram_tensor("v_band", [KBAND, D], FP, kind="ExternalInput")
    cm_d = nc.dram_tensor("colmask", [2, W], FP, kind="ExternalInput")
    mc_d = nc.dram_tensor("maskcol", [2, W], FP, kind="ExternalInput")
    out_d = nc.dram_tensor("out_rows", [HALF, D], FP, kind="ExternalOutput")
    mid_d = nc.dram_tensor("attn_mid", [HALF, HALF], FP, kind="ExternalOutput")
    far_d = nc.dram_tensor("attn_far", [HALF, HALF], FP, kind="ExternalOutput")
    msk_d = nc.dram_tensor("mask_rot", [512, S], U8, kind="ExternalOutput")

    def bcast(dram_handle, n_free):
        ap = dram_handle[:, :]
        return bass.AP(tensor=ap.tensor, offset=ap.offset, ap=[[0, 128]] + list(ap.ap))

    with tile.TileContext(nc) as tc:
        with (
            tc.tile_pool(name="consts", bufs=1) as consts,
            tc.tile_pool(name="big", bufs=1) as big,
            tc.tile_pool(name="sm", bufs=3) as sm,
            tc.tile_pool(name="stat", bufs=4) as stat,
            tc.tile_pool(name="ps", bufs=2, space="PSUM") as ps,
        ):
            ident = consts.tile([128, 128], FP, tag="ident")
            make_identity(nc, ident)

            # --- score band mask: 0 in band, NEG outside;  w' - p in [0, 64]
            bandmask = consts.tile([128, W], FP, tag="bandm")
            nc.vector.memset(bandmask, 0.0)
            nc.gpsimd.affine_select(
                out=bandmask, in_=bandmask, compare_op=ALU.is_ge, fill=NEG,
                base=0, channel_multiplier=-1, pattern=[[1, W]],
            )
            nc.gpsimd.affine_select(
                out=bandmask, in_=bandmask, compare_op=ALU.is_ge, fill=NEG,
                base=64, channel_multiplier=1, pattern=[[-1, W]],
            )
            cmb = consts.tile([128, 2, W], FP, tag="cmb")
            nc.gpsimd.dma_start(out=cmb, in_=bcast(cm_d, 2 * W))
            bm_first = consts.tile([128, W], FP, tag="bmf")
            nc.vector.tensor_add(bm_first, bandmask, cmb[:, 0, :])
            bm_last = consts.tile([128, W], FP, tag="bml")
            nc.vector.tensor_add(bm_last, bandmask, cmb[:, 1, :])

            # --- mask band pattern (1/0) in f32, clipped variants, cast to u8
            bandpat = consts.tile([128, W], FP, tag="bp")
            nc.vector.memset(bandpat, 1.0)
            nc.gpsimd.affine_select(
                out=bandpat, in_=bandpat, compare_op=ALU.is_ge, fill=0.0,
                base=0, channel_multiplier=-1, pattern=[[1, W]],
            )
            nc.gpsimd.affine_select(
                out=bandpat, in_=bandpat, compare_op=ALU.is_ge, fill=0.0,
                base=64, channel_multiplier=1, pattern=[[-1, W]],
            )
            mcb = consts.tile([128, 2, W], FP, tag="mcb")
            nc.gpsimd.dma_start(out=mcb, in_=bcast(mc_d, 2 * W))
            bp0f = consts.tile([128, W], FP, tag="bp0f")
            nc.vector.tensor_mul(bp0f, bandpat, mcb[:, 0, :])
            bp3f = consts.tile([128, W], FP, tag="bp3f")
            nc.vector.tensor_mul(bp3f, bandpat, mcb[:, 1, :])
            bp_u = consts.tile([128, W], U8, tag="bpu")
            nc.vector.tensor_copy(out=bp_u, in_=bandpat)
            bp0_u = consts.tile([128, W], U8, tag="bp0u")
            nc.vector.tensor_copy(out=bp0_u, in_=bp0f)
            bp3_u = consts.tile([128, W], U8, tag="bp3u")
            nc.vector.tensor_copy(out=bp3_u, in_=bp3f)

            # --- bulk loads (natural layout, partition = row % 128)
            q_all = big.tile([128, NBLK, D], FP, tag="qall")
            nc.scalar.dma_start(
                out=q_all, in_=q_d[:].rearrange("(m p) d -> p m d", p=128)
            )
            k_nat = big.tile([128, KTILES, D], FP, tag="knat")
            nc.scalar.dma_start(
                out=k_nat, in_=k_d[:].rearrange("(j p) d -> p j d", p=128)
            )
            v_sb = big.tile([128, KTILES, D], FP, tag="vsb")
            nc.scalar.dma_start(
                out=v_sb, in_=v_d[:].rearrange("(j p) d -> p j d", p=128)
            )

            # --- kT [d, row] via PE transposes
            kT = big.tile([128, KBAND], FP, tag="kT")
            for j in range(KTILES):
                tp = ps.tile([128, 128], FP, tag="tp")
                nc.tensor.transpose(tp, k_nat[:, j, :], ident)
                nc.vector.tensor_copy(out=kT[:, 128 * j : 128 * (j + 1)], in_=tp)

            # --- mask output (4 blocks, band at rot cols [128tb, 128tb+192))
            for tb in range(4):
                mt = big.tile([128, S], U8, tag=f"mask{tb}")
                nc.vector.memset(mt, 0)
                src = bp0_u if tb == 0 else (bp3_u if tb == 3 else bp_u)
                nc.gpsimd.tensor_copy(out=mt[:, 128 * tb : 128 * tb + W], in_=src)
                nc.sync.dma_start(out=msk_d[128 * tb : 128 * (tb + 1), :], in_=mt)

            # --- zero / fringe tiles for the far half
            zero_mid = big.tile([128, HALF], FP, tag="zm")
            nc.vector.memset(zero_mid, 0.0)
            far0 = big.tile([128, HALF], FP, tag="f0")
            nc.vector.memset(far0, 0.0)
            far15 = big.tile([128, HALF], FP, tag="f15")
            nc.vector.memset(far15, 0.0)

            rts = []
            for i in range(NB):
                rt = big.tile([128, HALF], FP, tag=f"rt{i}")
                nc.vector.memset(rt, 0.0)
                rts.append(rt)

            o_acc = big.tile([128, NBLK, D], FP, tag="oacc")

            for m in range(NBLK):
                qt_ps = ps.tile([128, 128], FP, tag="tp")
                nc.tensor.transpose(qt_ps, q_all[:, m, :], ident)
                qT = sm.tile([128, 128], FP, tag="qT")
                nc.vector.tensor_copy(out=qT, in_=qt_ps)

                sc_ps = ps.tile([128, W], FP, tag="scps")
                nc.tensor.matmul(
                    sc_ps, qT, kT[:, 128 * m : 128 * m + W], start=True, stop=True
                )
                bm = bm_first if m == 0 else (bm_last if m == 15 else bandmask)
                s_sb = sm.tile([128, W], FP, tag="s")
                nc.vector.scalar_tensor_tensor(
                    out=s_sb, in0=sc_ps, scalar=SCALE, in1=bm,
                    op0=ALU.mult, op1=ALU.add,
                )
                negmax = stat.tile([128, 1], FP, tag="negmax")
                nc.vector.tensor_reduce(
                    negmax, s_sb, mybir.AxisListType.X, ALU.max, negate=True
                )
                p_sb = sm.tile([128, W], FP, tag="p")
                denom = stat.tile([128, 1], FP, tag="den")
                nc.scalar.activation(
                    out=p_sb, in_=s_sb, func=AF.Exp,
                    bias=negmax[:, 0:1], scale=1.0, accum_out=denom,
                )
                rden = stat.tile([128, 1], FP, tag="rden")
                nc.vector.reciprocal(rden, denom)
                w_sb = sm.tile([128, W], FP, tag="w")
                nc.vector.tensor_scalar_mul(w_sb, p_sb, rden[:, 0:1])

                rt = rts[m % NB]
                if m == 0:
                    nc.gpsimd.tensor_copy(out=rt[:, 0:160], in_=w_sb[:, 32:192])
                    nc.gpsimd.tensor_copy(
                        out=far0[0:32, 2016:2048], in_=w_sb[0:32, 0:32]
                    )
                elif m == 15:
                    nc.gpsimd.tensor_copy(out=rt[:, 1888:2048], in_=w_sb[:, 0:160])
                    nc.gpsimd.tensor_copy(
                        out=far15[96:128, 0:32], in_=w_sb[96:128, 160:192]
                    )
                else:
                    nc.gpsimd.tensor_copy(
                        out=rt[:, 128 * m - 32 : 128 * m + 160], in_=w_sb
                    )
                nc.sync.dma_start(out=mid_d[128 * m : 128 * (m + 1), :], in_=rt)
                lo, hi = max(0, 128 * m - 32), min(HALF, 128 * m + 160)
                nc.vector.memset(rt[:, lo:hi], 0.0)

                if m == 0:
                    nc.sync.dma_start(out=far_d[0:128, :], in_=far0)
                elif m == 15:
                    nc.sync.dma_start(out=far_d[1920:2048, :], in_=far15)
                else:
                    nc.sync.dma_start(
                        out=far_d[128 * m : 128 * (m + 1), :], in_=zero_mid
                    )

                # attention output: O = w @ v_band  (contract over band rows)
                wta_ps = ps.tile([128, 128], FP, tag="tp")
                nc.tensor.transpose(wta_ps, w_sb[:, 0:128], ident)
                wta = sm.tile([128, 128], FP, tag="wta")
                nc.vector.tensor_copy(out=wta, in_=wta_ps)
                wtb_ps = ps.tile([64, 128], FP, tag="tpb")
                nc.tensor.transpose(wtb_ps, w_sb[:, 128:W], ident)
                wtb = sm.tile([64, 128], FP, tag="wtb")
                nc.vector.tensor_copy(out=wtb, in_=wtb_ps)
                o_ps = ps.tile([128, 128], FP, tag="ops")
                nc.tensor.matmul(o_ps, wta, v_sb[:, m, :], start=True, stop=False)
                nc.tensor.matmul(
                    o_ps, wtb[0:64, :], v_sb[0:64, m + 1, :], start=False, stop=True
                )
                nc.vector.tensor_copy(out=o_acc[:, m, :], in_=o_ps)

            nc.sync.dma_start(
                out=out_d[:].rearrange("(m p) d -> p m d", p=128), in_=o_acc
            )

    nc.compile()
    return nc


def kernel(q, k, v):
    q = np.asarray(q, np.float32)
    k = np.asarray(k, np.float32)
    v = np.asarray(v, np.float32)
    in_maps = _shards(q, k, v)

    if os.environ.get("KERNEL_EMULATE", "0") == "1":
        results = [_emulate_core(m) for m in in_maps]
        return _assemble(results)

    from concourse.bass_utils import run_bass_kernel_spmd

    if "nc" not in _CACHE:
        _CACHE["nc"] = _build_nc()
    res = run_bass_kernel_spmd(
        _CACHE["nc"],
        in_maps,
        core_ids=list(range(8)),
        trace=os.environ.get("KERNEL_TRACE", "0") == "1",
    )
    _CACHE["last"] = res
    return _assemble(res.results)


# revision 6
# speedup vs baseline: 251.2702x; 251.2702x over previous
"""Local sparse (sliding-window) attention for Trainium2, 8 NeuronCores.

Problem: q,k,v [4, 4096, 128] f32; window |i-j| <= 32.
Reference returns (output [4,4096,128], attn_weights [4,4096,4096], local_mask [4096,4096] bool).

Sharding (SPMD, one NEFF on 8 cores; all per-core differences are carried in
input *values*, never in code/offsets):
  core c -> batch b = c//2, query rows [base, base+2048) with base = 2048*(c%2).
  Within a 128-row query block m, the +-32 band covers a 192-wide column slab
  at columns base + 128m - 32 ... +160.  Relative to the core's own column
  half [base, base+2048) the slab offset 128m - 32 is core-INDEPENDENT, so:
    - attn_mid [2048, 2048]: the core's near column half (zeros + band slabs).
    - attn_far [2048, 2048]: the other column half: zeros + two 32x32 fringe
      corners (band spill across the half boundary).  Both fringe positions
      are static; the invalid one is exactly zero via the boundary col-mask.
    - out_rows [2048, 128]: attention output rows.
    - mask_rot [512, 4096] u8: local_mask rows [512c, 512c+512), cyclically
      rotated left by (512c - 32) mod 4096 so the band sits at columns [0,576)
      for every core; the host un-rotates with two slice copies.
"""

import os
import sys

import numpy as np

if "/opt/trn_rl_repo" not in sys.path:
    sys.path.insert(0, "/opt/trn_rl_repo")

B, S, D = 4, 4096, 128
HALF = 2048          # rows per core
NBLK = 16            # 128-row blocks per core
KTILES = 17          # padded k/v tiles of 128 rows (2176 rows)
KBAND = KTILES * 128
W = 192              # score slab width
WIN = 32             # half window
NB = 4               # row-tile ring size
SCALE = float(1.0 / np.sqrt(np.float32(D)))
NEG = -1.0e30

_CACHE = {}


# ---------------------------------------------------------------- host shards
def _shards(q, k, v):
    """Per-core input dicts (values differ per core, shapes identical)."""
    maps = []
    for c in range(8):
        b, base = c // 2, HALF * (c % 2)
        kb = np.zeros((KBAND, D), np.float32)
        vb = np.zeros((KBAND, D), np.float32)
        lo = base - WIN
        s_lo, s_hi = max(0, lo), min(S, lo + KBAND)
        kb[s_lo - lo : s_hi - lo] = k[b, s_lo:s_hi]
        vb[s_lo - lo : s_hi - lo] = v[b, s_lo:s_hi]
        # colmask[0/1, w]: additive mask for score slab of block 0 / block 15
        cm = np.zeros((2, W), np.float32)
        j0 = base - WIN + np.arange(W)            # abs col of block-0 slab
        cm[0][(j0 < 0) | (j0 >= S)] = NEG
        j15 = base + 15 * 128 - WIN + np.arange(W)
        cm[1][(j15 < 0) | (j15 >= S)] = NEG
        # maskcol[0/1, w]: validity (1/0) for mask-band blocks tb=0 / tb=3
        mc = np.ones((2, W), np.float32)
        a0 = 512 * c - WIN + np.arange(W)         # rot col w -> abs col
        mc[0][(a0 < 0) | (a0 >= S)] = 0.0
        a3 = 512 * c - WIN + 384 + np.arange(W)
        mc[1][(a3 < 0) | (a3 >= S)] = 0.0
        maps.append(
            {
                "q_rows": np.ascontiguousarray(q[b, base : base + HALF]),
                "k_band": kb,
                "v_band": vb,
                "colmask": cm,
                "maskcol": mc,
            }
        )
    return maps


def _assemble(results):
    out = np.empty((B, S, D), np.float32)
    attn = np.empty((B, S, S), np.float32)
    mask = np.empty((S, S), np.uint8)
    for c in range(8):
        r = results[c]
        b, base = c // 2, HALF * (c % 2)
        far = HALF - base
        out[b, base : base + HALF] = r["out_rows"]
        attn[b, base : base + HALF, base : base + HALF] = r["attn_mid"]
        attn[b, base : base + HALF, far : far + HALF] = r["attn_far"]
        rot = r["mask_rot"]
        a = (512 * c - WIN) % S
        mask[512 * c : 512 * c + 512, a:S] = rot[:, : S - a]
        mask[512 * c : 512 * c + 512, 0:a] = rot[:, S - a :]
    return out, attn, mask.view(np.bool_)


# ------------------------------------------------------- numpy emulation path
def _emulate_core(inp):
    """Numpy model of exactly what one core's device program computes."""
    qr, kb, vb = inp["q_rows"], inp["k_band"], inp["v_band"]
    cm, mc = inp["colmask"], inp["maskcol"]
    p_idx = np.arange(128)[:, None]
    w_idx = np.arange(W)[None, :]
    bandmask = np.where((w_idx - p_idx >= 0) & (w_idx - p_idx - 64 <= 0), 0.0, NEG)
    bandmask = bandmask.astype(np.float32)
    mid = np.zeros((HALF, HALF), np.float32)
    farr = np.zeros((HALF, HALF), np.float32)
    outr = np.empty((HALF, D), np.float32)
    for m in range(NBLK):
        qb = qr[128 * m : 128 * (m + 1)]
        kband = kb[128 * m : 128 * m + W]
        sc = qb @ kband.T
        bm = bandmask.copy()
        if m == 0:
            bm = bm + cm[0]
        if m == 15:
            bm = bm + cm[1]
        s = sc * np.float32(SCALE) + bm
        mx = s.max(axis=1, keepdims=True)
        p = np.exp(s - mx)
        wgt = (p / p.sum(axis=1, keepdims=True)).astype(np.float32)
        outr[128 * m : 128 * (m + 1)] = wgt @ vb[128 * m : 128 * m + W]
        if m == 0:
            mid[0:128, 0:160] = wgt[:, 32:192]
            farr[0:32, 2016:2048] = wgt[0:32, 0:32]
        elif m == 15:
            mid[1920:2048, 1888:2048] = wgt[:, 0:160]
            farr[2016:2048, 0:32] = wgt[96:128, 160:192]
        else:
            mid[128 * m : 128 * (m + 1), 128 * m - 32 : 128 * m + 160] = wgt
    # mask_rot: band pattern at cols [0, 576)
    rot = np.zeros((512, S), np.uint8)
    for tb in range(4):
        pat = ((w_idx - p_idx >= 0) & (w_idx - p_idx - 64 <= 0)).astype(np.uint8)
        if tb == 0:
            pat = pat * (mc[0] != 0)
        if tb == 3:
            pat = pat * (mc[1] != 0)
        rot[128 * tb : 128 * (tb + 1), 128 * tb : 128 * tb + W] = pat
    return {"out_rows": outr, "attn_mid": mid, "attn_far": farr, "mask_rot": rot}


# ------------------------------------------------------------- device program
def _build_nc():
    import concourse.bacc as bacc
    import concourse.bass as bass
    import concourse.mybir as mybir
    import concourse.tile as tile
    from concourse._compat import get_trn_type
    from concourse.masks import make_identity

    FP = mybir.dt.float32
    U8 = mybir.dt.uint8
    ALU = mybir.AluOpType
    AF = mybir.ActivationFunctionType

    nc = bacc.Bacc(get_trn_type() or "TRN2", target_bir_lowering=False, debug=False)
    q_d = nc.dram_tensor("q_rows", [HALF, D], FP, kind="ExternalInput")
    k_d = nc.dram_tensor("k_band", [KBAND, D], FP, kind="ExternalInput")
    v_d = nc.dram_tensor("v_band", [KBAND, D], FP, kind="ExternalInput")
    cm_d = nc.dram_tensor("colmask", [2, W], FP, kind="ExternalInput")
    mc_d = nc.dram_tensor("maskcol", [2, W], FP, kind="ExternalInput")
    out_d = nc.dram_tensor("out_rows", [HALF, D], FP, kind="ExternalOutput")
    mid_d = nc.dram_tensor("attn_mid", [HALF, HALF], FP, kind="ExternalOutput")
    far_d = nc.dram_tensor("attn_far", [HALF, HALF], FP, kind="ExternalOutput")
    msk_d = nc.dram_tensor("mask_rot", [512, S], U8, kind="ExternalOutput")

    def bcast(dram_handle, n_free):
        ap = dram_handle[:, :]
        return bass.AP(tensor=ap.tensor, offset=ap.offset, ap=[[0, 128]] + list(ap.ap))

    with tile.TileContext(nc) as tc:
        with (
            tc.tile_pool(name="consts", bufs=1) as consts,
            tc.tile_pool(name="big", bufs=1) as big,
            tc.tile_pool(name="sm", bufs=4) as sm,
            tc.tile_pool(name="stat", bufs=4) as stat,
            tc.tile_pool(name="ps", bufs=2, space="PSUM") as ps,
        ):
            ident = consts.tile([128, 128], FP, tag="ident")
            make_identity(nc, ident)

            # --- score band mask: 0 in band, NEG outside;  w' - p in [0, 64]
            bandmask = consts.tile([128, W], FP, tag="bandm")
            nc.vector.memset(bandmask, 0.0)
            nc.gpsimd.affine_select(
                out=bandmask, in_=bandmask, compare_op=ALU.is_ge, fill=NEG,
                base=0, channel_multiplier=-1, pattern=[[1, W]],
            )
            nc.gpsimd.affine_select(
                out=bandmask, in_=bandmask, compare_op=ALU.is_ge, fill=NEG,
                base=64, channel_multiplier=1, pattern=[[-1, W]],
            )
            cmb = consts.tile([128, 2, W], FP, tag="cmb")
            nc.gpsimd.dma_start(out=cmb, in_=bcast(cm_d, 2 * W))
            bm_first = consts.tile([128, W], FP, tag="bmf")
            nc.vector.tensor_add(bm_first, bandmask, cmb[:, 0, :])
            bm_last = consts.tile([128, W], FP, tag="bml")
            nc.vector.tensor_add(bm_last, bandmask, cmb[:, 1, :])

            # --- mask band pattern (1/0) in f32, clipped variants, cast to u8
            bandpat = consts.tile([128, W], FP, tag="bp")
            nc.vector.memset(bandpat, 1.0)
            nc.gpsimd.affine_select(
                out=bandpat, in_=bandpat, compare_op=ALU.is_ge, fill=0.0,
                base=0, channel_multiplier=-1, pattern=[[1, W]],
            )
            nc.gpsimd.affine_select(
                out=bandpat, in_=bandpat, compare_op=ALU.is_ge, fill=0.0,
                base=64, channel_multiplier=1, pattern=[[-1, W]],
            )
            mcb = consts.tile([128, 2, W], FP, tag="mcb")
            nc.gpsimd.dma_start(out=mcb, in_=bcast(mc_d, 2 * W))
            bp0f = consts.tile([128, W], FP, tag="bp0f")
            nc.vector.tensor_mul(bp0f, bandpat, mcb[:, 0, :])
            bp3f = consts.tile([128, W], FP, tag="bp3f")
            nc.vector.tensor_mul(bp3f, bandpat, mcb[:, 1, :])
            bp_u = consts.tile([128, W], U8, tag="bpu")
            nc.vector.tensor_copy(out=bp_u, in_=bandpat)
            bp0_u = consts.tile([128, W], U8, tag="bp0u")
            nc.vector.tensor_copy(out=bp0_u, in_=bp0f)
            bp3_u = consts.tile([128, W], U8, tag="bp3u")
            nc.vector.tensor_copy(out=bp3_u, in_=bp3f)

            # --- bulk loads (natural layout, partition = row % 128)
            q_all = big.tile([128, NBLK, D], FP, tag="qall")
            nc.scalar.dma_start(
                out=q_all, in_=q_d[:].rearrange("(m p) d -> p m d", p=128)
            )
            k_nat = big.tile([128, KTILES, D], FP, tag="knat")
            nc.scalar.dma_start(
                out=k_nat, in_=k_d[:].rearrange("(j p) d -> p j d", p=128)
            )
            v_sb = big.tile([128, KTILES, D], FP, tag="vsb")
            nc.scalar.dma_start(
                out=v_sb, in_=v_d[:].rearrange("(j p) d -> p j d", p=128)
            )

            # --- kT [d, row] via PE transposes
            kT = big.tile([128, KBAND], FP, tag="kT")
            for j in range(KTILES):
                tp = ps.tile([128, 128], FP, tag="tp")
                nc.tensor.transpose(tp, k_nat[:, j, :], ident)
                nc.vector.tensor_copy(out=kT[:, 128 * j : 128 * (j + 1)], in_=tp)

            # --- mask output (4 blocks, band at rot cols [128tb, 128tb+192))
            for tb in range(4):
                mt = big.tile([128, S], U8, tag=f"mask{tb}")
                nc.vector.memset(mt, 0)
                src = bp0_u if tb == 0 else (bp3_u if tb == 3 else bp_u)
                nc.gpsimd.tensor_copy(out=mt[:, 128 * tb : 128 * tb + W], in_=src)
                nc.sync.dma_start(out=msk_d[128 * tb : 128 * (tb + 1), :], in_=mt)

            # --- zero / fringe tiles for the far half
            zero_mid = big.tile([128, HALF], FP, tag="zm")
            nc.vector.memset(zero_mid, 0.0)
            far0 = big.tile([128, HALF], FP, tag="f0")
            nc.vector.memset(far0, 0.0)
            far15 = big.tile([128, HALF], FP, tag="f15")
            nc.vector.memset(far15, 0.0)

            rts = []
            for i in range(NB):
                rt = big.tile([128, HALF], FP, tag=f"rt{i}")
                nc.vector.memset(rt, 0.0)
                rts.append(rt)

            for m in range(NBLK):
                qt_ps = ps.tile([128, 128], FP, tag="tp")
                nc.tensor.transpose(qt_ps, q_all[:, m, :], ident)
                qT = sm.tile([128, 128], FP, tag="qT")
                nc.vector.tensor_copy(out=qT, in_=qt_ps)

                sc_ps = ps.tile([128, W], FP, tag="scps")
                nc.tensor.matmul(
                    sc_ps, qT, kT[:, 128 * m : 128 * m + W], start=True, stop=True
                )
                bm = bm_first if m == 0 else (bm_last if m == 15 else bandmask)
                s_sb = sm.tile([128, W], FP, tag="s")
                nc.vector.scalar_tensor_tensor(
                    out=s_sb, in0=sc_ps, scalar=SCALE, in1=bm,
                    op0=ALU.mult, op1=ALU.add,
                )
                negmax = stat.tile([128, 1], FP, tag="negmax")
                nc.vector.tensor_reduce(
                    negmax, s_sb, mybir.AxisListType.X, ALU.max, negate=True
                )
                p_sb = sm.tile([128, W], FP, tag="p")
                denom = stat.tile([128, 1], FP, tag="den")
                nc.scalar.activation(
                    out=p_sb, in_=s_sb, func=AF.Exp,
                    bias=negmax[:, 0:1], scale=1.0, accum_out=denom,
                )
                rden = stat.tile([128, 1], FP, tag="rden")
                nc.vector.reciprocal(rden, denom)
                w_sb = sm.tile([128, W], FP, tag="w")
                nc.vector.tensor_scalar_mul(w_sb, p_sb, rden[:, 0:1])

                rt = rts[m % NB]
                if m == 0:
                    nc.gpsimd.tensor_copy(out=rt[:, 0:160], in_=w_sb[:, 32:192])
                    nc.gpsimd.tensor_copy(
                        out=far0[0:32, 2016:2048], in_=w_sb[0:32, 0:32]
                    )
                elif m == 15:
                    nc.gpsimd.tensor_copy(out=rt[:, 1888:2048], in_=w_sb[:, 0:160])
                    nc.gpsimd.tensor_copy(
                        out=far15[96:128, 0:32], in_=w_sb[96:128, 160:192]
                    )
                else:
                    nc.gpsimd.tensor_copy(
                        out=rt[:, 128 * m - 32 : 128 * m + 160], in_=w_sb
                    )
                nc.sync.dma_start(out=mid_d[128 * m : 128 * (m + 1), :], in_=rt)
                lo, hi = max(0, 128 * m - 32), min(HALF, 128 * m + 160)
                nc.vector.memset(rt[:, lo:hi], 0.0)

                if m == 0:
                    nc.sync.dma_start(out=far_d[0:128, :], in_=far0)
                elif m == 15:
                    nc.sync.dma_start(out=far_d[1920:2048, :], in_=far15)
                else:
                    nc.sync.dma_start(
                        out=far_d[128 * m : 128 * (m + 1), :], in_=zero_mid
                    )

                # attention output: O = w @ v_band  (contract over band rows)
                wta_ps = ps.tile([128, 128], FP, tag="tp")
                nc.tensor.transpose(wta_ps, w_sb[:, 0:128], ident)
                wta = sm.tile([128, 128], FP, tag="wta")
                nc.vector.tensor_copy(out=wta, in_=wta_ps)
                wtb_ps = ps.tile([64, 128], FP, tag="tpb")
                nc.tensor.transpose(wtb_ps, w_sb[:, 128:W], ident)
                wtb = sm.tile([64, 128], FP, tag="wtb")
                nc.vector.tensor_copy(out=wtb, in_=wtb_ps)
                o_ps = ps.tile([128, 128], FP, tag="ops")
                nc.tensor.matmul(o_ps, wta, v_sb[:, m, :], start=True, stop=False)
                nc.tensor.matmul(
                    o_ps, wtb[0:64, :], v_sb[0:64, m + 1, :], start=False, stop=True
                )
                o_sb = sm.tile([128, 128], FP, tag="osb")
                nc.vector.tensor_copy(out=o_sb, in_=o_ps)
                nc.sync.dma_start(out=out_d[128 * m : 128 * (m + 1), :], in_=o_sb)


    nc.compile()
    return nc


def kernel(q, k, v):
    q = np.asarray(q, np.float32)
    k = np.asarray(k, np.float32)
    v = np.asarray(v, np.float32)
    in_maps = _shards(q, k, v)

    if os.environ.get("KERNEL_EMULATE", "0") == "1":
        results = [_emulate_core(m) for m in in_maps]
        return _assemble(results)

    from concourse.bass_utils import run_bass_kernel_spmd

    if "nc" not in _CACHE:
        _CACHE["nc"] = _build_nc()
    res = run_bass_kernel_spmd(
        _CACHE["nc"],
        in_maps,
        core_ids=list(range(8)),
        trace=os.environ.get("KERNEL_TRACE", "0") == "1",
    )
    _CACHE["last"] = res
    return _assemble(res.results)
